# revision 15
# baseline (speedup 1.0000x reference)
"""Trainium2 Bass kernel for RecursiveMamba130M.

Math: the complex SSM state never materializes. With
  R = cos(theta) + j sin(theta),  Bc = Br + j Bi,  Cc = Cr + j Ci,
the per-loop output collapses to
  y_i[t, f] = sum_{k<=i} G_{i-k}[f] * u_k[t, f],   u_k = h_k @ W_in^T
where G_m[f] = sum_s Re(Cc * R^m * Bc).

The whole per-token scalar chain is commuted out of the PE's critical
path. With w = rs_z*z + h and h' = rs_w*w + step:
  u_{i+1} = (rs_z*rs_w)*q_i + rs_w*u_i + step@W_in^T,   q_i = z_i @ W_in^T
so the PE transposes z (available straight out of MM2) and runs
MM1 on it with NO dependency on the normalization scalars; the scalars
ride the PSUM evacuation (ACT per-partition scale) and a DVE fused op.
The second norm's statistics come from
  sum w^2 = rs_z^2*sum z^2 + 2*rs_z*sum(z*h) + sum h^2
so rs_w is ready before the evacuation needs it. h and w themselves are
materialized lazily on GpSimd, off every critical path. G0*step-terms
are folded into the accumulators as precomputed broadcast tiles.

Sharding: data-parallel over the 1024 positions (128 tokens/core, no
collectives); weights replicated, all matmul data bf16 (fp32 PSUM,
fp32 norm statistics). MM1 is chunk-major so each 512-wide PSUM chunk
retires early; MM2 runs as two accumulation groups (512/256) so the
wide chunk's norm partials hide under the narrow chunk's matmuls.
"""

import numpy as np
import ml_dtypes

import concourse.bass as bass
import concourse.tile as tile
from concourse.bacc import Bacc
from concourse import masks, mybir
from concourse.bass_utils import run_bass_kernel_spmd

T = 128          # tokens per core
D = 768          # d_model
F = 1536         # 2 * d_model
NL = 4           # reasoning loops
NCORES = 8
EPS = 1e-6

f32 = mybir.dt.float32
bf16 = mybir.dt.bfloat16
AL = mybir.AluOpType
AF = mybir.ActivationFunctionType

Z_CHUNKS = ((0, 512), (512, 256))
F_CHUNKS = ((0, 512), (512, 512), (1024, 512))

_CACHE = {}


def _act_rsqrt(nc, out, in_, bias_ap, scale):
    """out = Rsqrt(in_*scale + bias) on ScalarE (accuracy fine at 2e-2)."""
    eng = nc.scalar
    ins = [
        eng.lower_ap(in_),
        eng.lower_ap(bias_ap),
        mybir.ImmediateValue(dtype=mybir.dt.float32, value=float(scale)),
        mybir.ImmediateValue(dtype=mybir.dt.float32, value=0.0),
    ]
    return eng.add_instruction(
        mybir.InstActivation(
            name=nc.get_next_instruction_name(),
            func=AF.Rsqrt,
            ins=ins,
            outs=[eng.lower_ap(out)],
        )
    )


def _act_axpb(nc, out, in_, scale, bias_ap):
    """out = in_*scale + bias on ScalarE (Identity; scale may be an AP)."""
    return nc.scalar.activation(out, in_, AF.Identity, bias=bias_ap,
                                scale=scale)


def build_nc():
    nc = Bacc()
    x_d = nc.dram_tensor("x_in", [T, D], bf16, kind="ExternalInput")
    winT_d = nc.dram_tensor("winT", [D, F], bf16, kind="ExternalInput")
    woutT_d = nc.dram_tensor("woutT", [F, D], bf16, kind="ExternalInput")
    # rows_f: [0:4]=G rows, [4:8]=sW rows, [8:12]=G0*sW rows
    rowsf_d = nc.dram_tensor("rows_f", [3 * NL, F], bf16,
                             kind="ExternalInput")
    s4_d = nc.dram_tensor("s4", [NL, D], bf16, kind="ExternalInput")
    out_d = nc.dram_tensor("x_out", [T, D], f32, kind="ExternalOutput")

    with tile.TileContext(nc) as tc:
        with (
            tc.tile_pool(name="wpool", bufs=1) as wpool,
            tc.tile_pool(name="apool", bufs=1) as apool,
            tc.tile_pool(name="work", bufs=2) as work,
            tc.tile_pool(name="scal", bufs=1) as scal,
            tc.tile_pool(name="ps_u", bufs=1, space="PSUM") as ps_u,
            tc.tile_pool(name="ps_z", bufs=1, space="PSUM") as ps_z,
            tc.tile_pool(name="ps_t", bufs=1, space="PSUM") as ps_t,
            tc.tile_pool(name="ps_y", bufs=1, space="PSUM") as ps_y,
        ):
            # ---------- constants ----------
            ident = wpool.tile([128, 128], bf16, tag="ident")
            masks.make_identity(nc, ident[:])
            ones1 = wpool.tile([1, 128], bf16, tag="ones1")
            nc.vector.memset(ones1[:].bitcast(mybir.dt.uint32), 0x3F803F80)
            eps_t = wpool.tile([T, 1], f32, tag="eps_t")
            nc.vector.memset(eps_t[:], EPS)

            # ---------- DMAs (order = priority) ----------
            x_sb = wpool.tile([T, D], bf16, tag="x_sb")
            nc.sync.dma_start(x_sb[:], x_d[:, :])
            rows_f = wpool.tile([1, 3 * NL, F], bf16, tag="rows_f")
            nc.sync.dma_start(rows_f[:], rowsf_d.rearrange("r n -> () r n"))
            rows_s = wpool.tile([1, NL, D], bf16, tag="rows_s")
            nc.sync.dma_start(rows_s[:], s4_d.rearrange("r n -> () r n"))

            winT_sb = wpool.tile([128, 6, F], bf16, tag="winT_sb")
            for k in range(6):
                nc.sync.dma_start(winT_sb[:, k, :],
                                  winT_d[128 * k:128 * (k + 1), :])
            woutT_sb = wpool.tile([128, 12, D], bf16, tag="woutT_sb")
            for g in range(4):
                nc.sync.dma_start(
                    woutT_sb[:, 3 * g:3 * (g + 1), :],
                    woutT_d[384 * g:384 * (g + 1), :].rearrange(
                        "(k p) n -> p k n", p=128))

            # ---------- broadcast tiles via K=1 ones-matmul ----------
            def bcast_build(dst, row_ap, chunks, eng_copy, label):
                for ci, (off, nn) in enumerate(chunks):
                    if nn == 512:
                        pt = ps_u.tile([T, 512], f32, tag=f"u{ci}",
                                       name=f"bc_{label}_{ci}")
                    else:
                        pt = ps_z.tile([T, nn], f32, tag=f"z{ci}",
                                       name=f"bc_{label}_{ci}")
                    nc.tensor.matmul(pt[:], ones1[:, :],
                                     row_ap[:, off:off + nn],
                                     start=True, stop=True)
                    eng_copy(dst[:, off:off + nn], pt[:])

            # Gb[m], sWb[i] (i>=1), gswb[i] (= G0*sW_i, i>=1), Sb[i]
            Sb, Gb, sWb, gswb = [], [], [None], [None]
            for i in range(NL):
                sb = wpool.tile([T, D], bf16, tag=f"Sb{i}", name=f"Sb{i}")
                eng = nc.vector.tensor_copy if i == 0 else nc.scalar.copy
                bcast_build(sb, rows_s[:, i, :], Z_CHUNKS, eng, f"sb{i}")
                Sb.append(sb)
            for m in range(NL):
                gb = wpool.tile([T, F], bf16, tag=f"Gb{m}", name=f"Gb{m}")
                eng = (nc.vector.tensor_copy if m in (0, 1)
                       else nc.scalar.copy)
                bcast_build(gb, rows_f[:, m, :], F_CHUNKS, eng, f"gb{m}")
                Gb.append(gb)
            for i in range(1, NL):
                swt = wpool.tile([T, F], bf16, tag=f"sWb{i}", name=f"sWb{i}")
                bcast_build(swt, rows_f[:, NL + i, :], F_CHUNKS,
                            nc.vector.tensor_copy, f"sw{i}")
                sWb.append(swt)
                gst = wpool.tile([T, F], bf16, tag=f"gswb{i}",
                                 name=f"gswb{i}")
                bcast_build(gst, rows_f[:, 2 * NL + i, :], F_CHUNKS,
                            nc.scalar.copy, f"gs{i}")
                gswb.append(gst)

            # ---------- h0 = x + Sb0; transpose h0 on PE ----------
            h = work.tile([T, D], bf16, tag="h", bufs=2)
            nc.vector.tensor_add(h[:], x_sb[:], Sb[0][:])
            ssh = scal.tile([T, 1], f32, tag="ssh", bufs=2)
            scr0 = work.tile([T, D], bf16, tag="scr", bufs=2)
            nc.scalar.activation(scr0[:], h[:], AF.Square, accum_out=ssh[:])

            def transpose_to_sbuf(src, label, splits):
                """PE-transpose src [T, 768] -> [T, 768] bf16 k-tile major,
                one-bank bf16 PSUM, DVE evacuation per 3-tile half."""
                t_ps = ps_t.tile([T, 6, 128], bf16, tag="t",
                                 name=f"tps_{label}")
                for lo, hi in splits:
                    for k in range(lo, hi):
                        nc.tensor.transpose(
                            t_ps[:, k, :],
                            src[:, 128 * k:128 * (k + 1)],
                            ident[:],
                        )
                t_sb = work.tile([T, D], bf16, tag="pT_sb", bufs=2,
                                 name=f"tsb_{label}")
                for half in range(2):
                    nc.vector.tensor_copy(
                        t_sb[:, 384 * half:384 * (half + 1)],
                        t_ps[:, 3 * half:3 * (half + 1), :])
                return t_sb

            pT_sb = transpose_to_sbuf(h, "h0", ((0, 6),))

            accs = {}
            for j in (1, 2, 3):
                accs[j] = apool.tile([T, F], bf16, tag=f"acc{j}",
                                     name=f"acc{j}")

            rs_w = None   # rs_w of loop i-1
            s1 = None     # rs_z*rs_w of loop i-1
            u_prev = None  # true u_{i-1} (bf16 SBUF)
            # ---------- main loop ----------
            for i in range(NL):
                # MM1: q = pT @ W_in^T  (= u_0 at i=0, else z_{i-1} @ W_in^T)
                # Loop 0 is winT-DMA-paced: k-major. Steady: chunk-major.
                u_ps = [ps_u.tile([T, 512], f32, tag=f"u{n}",
                                  name=f"u{i}_{n}") for n in range(3)]
                mm1_order = (
                    [(n, k) for k in range(6) for n in range(3)] if i == 0
                    else [(n, k) for n in range(3) for k in range(6)])
                for n, k in mm1_order:
                    nc.tensor.matmul(
                        u_ps[n][:],
                        pT_sb[:, 128 * k:128 * (k + 1)],
                        winT_sb[:, k, 512 * n:512 * (n + 1)],
                        start=(k == 0), stop=(k == 5),
                    )

                # Evacuate with the s1 scale (i>0): p = s1*q; then the y
                # path reads p directly (step-constant is pre-folded into
                # acc), so y needs only mul+add after the evacuation.
                p_sb = work.tile([T, F], bf16, tag="p_sb", bufs=2)
                y = work.tile([T, F], bf16, tag="y", bufs=2)
                yT_ps = ps_y.tile([T, 12, 128], bf16, tag="yt")
                yT_sb = work.tile([128, 12, 128], bf16, tag="yT_sb", bufs=2)
                for n in range(3):
                    sl = slice(512 * n, 512 * (n + 1))
                    if i == 0:
                        nc.scalar.copy(p_sb[:, sl], u_ps[n][:])
                        nc.vector.tensor_mul(y[:, sl], p_sb[:, sl],
                                             Gb[0][:, sl])
                    else:
                        nc.scalar.activation(p_sb[:, sl], u_ps[n][:],
                                             AF.Copy, scale=s1[:, :])
                        # y = G0*(s1*q + rs_w*u_prev) + acc''
                        #   = G0*p + G0*rs_w*u_prev + acc''; the middle term
                        #   rides the u-recursion: do mul on the recursion
                        #   partial instead. Build the true-u partial first:
                        nc.vector.scalar_tensor_tensor(
                            out=p_sb[:, sl], in0=u_prev[:, sl],
                            scalar=rs_w[:, :], in1=p_sb[:, sl],
                            op0=AL.mult, op1=AL.add,
                        )
                        nc.vector.tensor_mul(y[:, sl], p_sb[:, sl],
                                             Gb[0][:, sl])
                        nc.vector.tensor_add(y[:, sl], y[:, sl],
                                             accs[i][:, sl])
                    for c in range(4 * n, 4 * (n + 1)):
                        nc.tensor.transpose(
                            yT_ps[:, c, :],
                            y[:, 128 * c:128 * (c + 1)],
                            ident[:],
                        )
                    nc.vector.tensor_copy(yT_sb[:, 4 * n:4 * (n + 1), :],
                                          yT_ps[:, 4 * n:4 * (n + 1), :])

                # true u_i = p + sWb_i (off the y path; consumed by the
                # next loop's recursion and the lag products)
                if i == 0:
                    u_true = p_sb
                else:
                    u_true = work.tile([T, F], bf16, tag="u_true", bufs=2)
                    nc.vector.tensor_add(u_true[:], p_sb[:], sWb[i][:])

                # MM2: z = y @ out_proj^T, A(512) then B(256) groups
                z_ps = []
                for ci, (off, nn) in enumerate(Z_CHUNKS):
                    zt = ps_z.tile([T, nn], f32, tag=f"z{ci}",
                                   name=f"z{i}_{ci}")
                    for c in range(12):
                        nc.tensor.matmul(
                            zt[:],
                            yT_sb[:, c, :],
                            woutT_sb[:, c, off:off + nn],
                            start=(c == 0), stop=(c == 11),
                        )
                    z_ps.append(zt)
                    # Square partial right behind the chunk's stop (ACT);
                    # the A partial runs under B's matmuls
                    ssp = scal.tile([T, 1], f32, tag=f"ssz{ci}")
                    scr = work.tile([T, 512], bf16, tag="scr5", bufs=2)
                    nc.scalar.activation(scr[:, 0:nn], zt[:], AF.Square,
                                         accum_out=ssp[:])
                    if ci == 0:
                        ssz_A = ssp
                    else:
                        ss_z = scal.tile([T, 1], f32, tag="ss_z")
                        _act_axpb(nc, ss_z[:], ssz_A[:], 1.0, ssp[:, :])
                # z*h partials for the rs_w identity (DVE, PSUM source)
                szh_p = []
                for ci, (off, nn) in enumerate(Z_CHUNKS):
                    shp = scal.tile([T, 1], f32, tag=f"szh{ci}")
                    zscr = work.tile([T, 512], bf16, tag="zscr", bufs=2)
                    nc.vector.scalar_tensor_tensor(
                        out=zscr[:, 0:nn], in0=z_ps[ci][:], scalar=2.0,
                        in1=h[:, off:off + nn], op0=AL.mult, op1=AL.mult,
                        accum_out=shp[:],
                    )
                    szh_p.append(shp)
                szh2 = scal.tile([T, 1], f32, tag="szh2")
                _act_axpb(nc, szh2[:], szh_p[0][:], 1.0, szh_p[1][:, :])

                rs_z = scal.tile([T, 1], f32, tag="rs_z", bufs=2,
                                 name=f"rs_z{i}")
                _act_rsqrt(nc, rs_z[:], ss_z[:], eps_t[:, :], 1.0 / D)

                if i < NL - 1:
                    # z -> SBUF bf16 (chunk A rides under B's matmuls),
                    # then PE-transpose z for the next loop's MM1
                    z_sb = work.tile([T, D], bf16, tag="z_sb", bufs=2)
                    nc.vector.tensor_copy(z_sb[:, 0:512], z_ps[0][:])
                    nc.vector.tensor_copy(z_sb[:, 512:768], z_ps[1][:])
                    pT_sb = transpose_to_sbuf(z_sb, f"z{i}",
                                              ((0, 4), (4, 6)))

                # ss_w identity: rs_w must beat the next loop's evacuation
                v1 = scal.tile([T, 1], f32, tag="v1")
                _act_axpb(nc, v1[:], ss_z[:], rs_z[:, :], szh2[:, :])
                ss_w = scal.tile([T, 1], f32, tag="ss_w")
                _act_axpb(nc, ss_w[:], v1[:], rs_z[:, :], ssh[:, :])
                rs_w = scal.tile([T, 1], f32, tag="rs_w", bufs=2,
                                 name=f"rs_w{i}")
                _act_rsqrt(nc, rs_w[:], ss_w[:], eps_t[:, :], 1.0 / D)
                s1 = scal.tile([T, 1], f32, tag="s1", bufs=2,
                               name=f"s1_{i}")
                nc.vector.tensor_mul(s1[:], rs_z[:], rs_w[:])

                # lag products into accumulators (muls on DVE, the
                # cross-loop += on GpSimd; all deep-slack)
                for j in range(i + 1, NL):
                    m = j - i
                    if i == 0:
                        nc.vector.tensor_mul(accs[j][:], u_true[:],
                                             Gb[m][:])
                    else:
                        tmp_a = work.tile([T, F], bf16, tag="tmp_a", bufs=2)
                        nc.vector.tensor_mul(tmp_a[:], u_true[:], Gb[m][:])
                        nc.gpsimd.tensor_add(accs[j][:], accs[j][:],
                                             tmp_a[:])
                    if j == i + 1:
                        # fold the G0*sW_j constant now: y_j reads p (sans
                        # step), so acc_j must carry G0*sW_j
                        nc.gpsimd.tensor_add(accs[j][:], accs[j][:],
                                             gswb[j][:])

                # h/w materialization, off every critical path (GpSimd)
                if i < NL - 1:
                    w = work.tile([T, D], bf16, tag="w", bufs=2)
                    nc.vector.scalar_tensor_tensor(
                        out=w[:], in0=z_sb[:], scalar=rs_z[:, :], in1=h[:],
                        op0=AL.mult, op1=AL.add,
                    )
                    h_next = work.tile([T, D], bf16, tag="h", bufs=2)
                    nc.vector.scalar_tensor_tensor(
                        out=h_next[:], in0=w[:], scalar=rs_w[:, :],
                        in1=Sb[i + 1][:], op0=AL.mult, op1=AL.add,
                    )
                    h = h_next
                    ssh = scal.tile([T, 1], f32, tag="ssh", bufs=2)
                    scrh = work.tile([T, D], bf16, tag="scr", bufs=2)
                    nc.scalar.activation(scrh[:], h[:], AF.Square,
                                         accum_out=ssh[:])
                    u_prev = u_true
                else:
                    # final output: x = rs_w * (rs_z*z + h), fp32
                    w3 = work.tile([T, D], bf16, tag="w", bufs=2)
                    for ci, (off, nn) in enumerate(Z_CHUNKS):
                        nc.vector.scalar_tensor_tensor(
                            out=w3[:, off:off + nn], in0=z_ps[ci][:],
                            scalar=rs_z[:, :], in1=h[:, off:off + nn],
                            op0=AL.mult, op1=AL.add,
                        )
                    out_f = work.tile([T, D], f32, tag="out_f", bufs=1)
                    nc.vector.tensor_scalar_mul(out_f[:], w3[:], rs_w[:, :])
                    nc.sync.dma_start(out_d[:, :], out_f[:])

    nc.compile()
    return nc


def _host_prep(x, in_proj_base, lora_A, lora_B, A_theta, B_real, B_imag,
               C_real, C_imag, out_proj_w, step_emb):
    W_in = in_proj_base.astype(np.float64) + 2.0 * (
        lora_B.astype(np.float64) @ lora_A.astype(np.float64))
    winT = np.ascontiguousarray(W_in.T).astype(ml_dtypes.bfloat16)
    woutT = np.ascontiguousarray(out_proj_w.T).astype(ml_dtypes.bfloat16)

    th = A_theta.astype(np.float64)
    P = (C_real.astype(np.float64) * B_real.astype(np.float64)
         - C_imag.astype(np.float64) * B_imag.astype(np.float64))
    Q = (C_real.astype(np.float64) * B_imag.astype(np.float64)
         + C_imag.astype(np.float64) * B_real.astype(np.float64))
    g4 = np.stack([
        (P * np.cos(m * th) - Q * np.sin(m * th)).sum(-1).reshape(-1)
        for m in range(NL)
    ])                                                       # [4, 1536]
    sW = step_emb.astype(np.float64) @ W_in.T                # [4, F]
    sW[0] = 0.0   # h0 = x + s0 handled explicitly
    gsw = g4[0][None, :] * sW                                # G0*sW rows
    rows_f = np.concatenate([g4, sW, gsw], 0)                # [12, F]
    s4 = np.ascontiguousarray(step_emb).astype(ml_dtypes.bfloat16)
    return winT, woutT, rows_f.astype(ml_dtypes.bfloat16), s4


def kernel(x, in_proj_base, lora_A, lora_B, A_theta, B_real, B_imag,
           C_real, C_imag, out_proj_w, mixer_norm_w, loop_norm_w, step_emb,
           _trace=False):
    x = np.asarray(x, dtype=np.float32)
    winT, woutT, rows_f, s4 = _host_prep(
        np.asarray(x), np.asarray(in_proj_base), np.asarray(lora_A),
        np.asarray(lora_B), np.asarray(A_theta), np.asarray(B_real),
        np.asarray(B_imag), np.asarray(C_real), np.asarray(C_imag),
        np.asarray(out_proj_w), np.asarray(step_emb))
    # mixer_norm_w / loop_norm_w are ones per the problem spec; rmsnorm weight
    # multiplies are identity and omitted on device.

    if "nc" not in _CACHE:
        _CACHE["nc"] = build_nc()
    nc = _CACHE["nc"]

    x16 = x.astype(ml_dtypes.bfloat16)
    shared = {"winT": winT, "woutT": woutT, "rows_f": rows_f, "s4": s4}
    in_maps = [
        {**shared, "x_in": np.ascontiguousarray(x16[0, T * c:T * (c + 1), :])}
        for c in range(NCORES)
    ]
    res = run_bass_kernel_spmd(nc, in_maps, list(range(NCORES)), trace=_trace)
    out = np.concatenate(
        [np.asarray(res.results[c]["x_out"]) for c in range(NCORES)], axis=0)
    if _trace:
        _CACHE["last_result"] = res
    return out[None, :, :].astype(np.float32)


# revision 18
# speedup vs baseline: 1.3029x; 1.3029x over previous
"""Trainium2 Bass kernel for RecursiveMamba130M.

Math: the complex SSM state never materializes. With
  R = cos(theta) + j sin(theta),  Bc = Br + j Bi,  Cc = Cr + j Ci,
the per-loop output collapses to
  y_i[t, f] = sum_{k<=i} G_{i-k}[f] * u_k[t, f],   u_k = h_k @ W_in^T
where G_m[f] = sum_s Re(Cc * R^m * Bc).

Algebraic folds that keep the PE dense:
  * h_{i+1} = rs_w*w + step  =>  u_{i+1} = rs_w*(w @ W_in^T) + step@W_in^T.
    The per-token scale rs_w commutes through the matmul, so the PE
    transposes w (available right after rs_z) instead of h, and the
    rs_w scale rides the ACT PSUM->SBUF evacuation for free.
  * The step@W_in^T terms are constant rows; their contribution to z is
    zdb_i = (sum_k G_{i-k}*sW_k) @ W_out^T, injected into MM2's PSUM
    accumulation as a rank-1 ones-matmul. y on device is pure
    G0*u' + acc.

Engine budget per loop: PE 42 matmuls + 18 transposes; ACT evacuates
u' (with the rs_w scale) and the transposed w, and runs the Rsqrts;
DVE does the y combine, the norm statistics (fused square+accumulate)
and w; GpSimd takes the deep-slack accumulator adds. A burst of
no-dependency warm-up matmuls at kernel start brings the PE clock to
2.4 GHz while the weight DMA streams in.

Sharding: data-parallel over the 1024 positions (128 tokens/core, no
collectives); weights replicated, all matmul data bf16 (fp32 PSUM,
fp32 norm statistics). MM1 is chunk-major so each 512-wide PSUM chunk
retires early; MM2 runs as two accumulation groups (512/256) so the
wide chunk's norm partials hide under the narrow chunk's matmuls.
"""

import numpy as np
import ml_dtypes

import concourse.bass as bass
import concourse.tile as tile
from concourse.bacc import Bacc
from concourse import masks, mybir
from concourse.bass_utils import run_bass_kernel_spmd

T = 128          # tokens per core
D = 768          # d_model
F = 1536         # 2 * d_model
NL = 4           # reasoning loops
NCORES = 8
EPS = 1e-6

f32 = mybir.dt.float32
bf16 = mybir.dt.bfloat16
AL = mybir.AluOpType
AF = mybir.ActivationFunctionType

Z_CHUNKS = ((0, 512), (512, 256))
F_CHUNKS = ((0, 512), (512, 512), (1024, 512))

_CACHE = {}


def _act_rsqrt(nc, out, in_, bias_ap, scale):
    """out = Rsqrt(in_*scale + bias) on ScalarE (accuracy fine at 2e-2)."""
    eng = nc.scalar
    ins = [
        eng.lower_ap(in_),
        eng.lower_ap(bias_ap),
        mybir.ImmediateValue(dtype=mybir.dt.float32, value=float(scale)),
        mybir.ImmediateValue(dtype=mybir.dt.float32, value=0.0),
    ]
    return eng.add_instruction(
        mybir.InstActivation(
            name=nc.get_next_instruction_name(),
            func=AF.Rsqrt,
            ins=ins,
            outs=[eng.lower_ap(out)],
        )
    )


def build_nc():
    nc = Bacc()
    x_d = nc.dram_tensor("x_in", [T, D], bf16, kind="ExternalInput")
    winT_d = nc.dram_tensor("winT", [D, F], bf16, kind="ExternalInput")
    woutT_d = nc.dram_tensor("woutT", [F, D], bf16, kind="ExternalInput")
    g4_d = nc.dram_tensor("g4", [NL, F], bf16, kind="ExternalInput")
    s4_d = nc.dram_tensor("s4", [NL, D], bf16, kind="ExternalInput")
    zdb_d = nc.dram_tensor("zdb", [NL, D], bf16, kind="ExternalInput")
    out_d = nc.dram_tensor("x_out", [T, D], f32, kind="ExternalOutput")

    with tile.TileContext(nc) as tc:
        with (
            tc.tile_pool(name="wpool", bufs=1) as wpool,
            tc.tile_pool(name="apool", bufs=1) as apool,
            tc.tile_pool(name="work", bufs=2) as work,
            tc.tile_pool(name="scal", bufs=1) as scal,
            tc.tile_pool(name="ps_u", bufs=1, space="PSUM") as ps_u,
            tc.tile_pool(name="ps_z", bufs=1, space="PSUM") as ps_z,
            tc.tile_pool(name="ps_t", bufs=1, space="PSUM") as ps_t,
            tc.tile_pool(name="ps_y", bufs=1, space="PSUM") as ps_y,
        ):
            # ---------- constants ----------
            ident = wpool.tile([128, 128], bf16, tag="ident")
            masks.make_identity(nc, ident[:])
            ones1 = wpool.tile([1, 128], bf16, tag="ones1")
            nc.vector.memset(ones1[:].bitcast(mybir.dt.uint32), 0x3F803F80)
            eps_t = wpool.tile([T, 1], f32, tag="eps_t")
            nc.vector.memset(eps_t[:], EPS)

            # PE warm-up: ~3.5us of no-dependency matmuls while the weight
            # DMA streams in, so the activity monitor lifts the clock gate
            # to 2.4 GHz before the first real matmul
            warm_ps = ps_u.tile([T, 512], f32, tag="u0", name="warm")
            for _ in range(25):
                nc.tensor.matmul(warm_ps[:, 0:128], ident[:, :], ident[:, :],
                                 start=True, stop=True)

            # ---------- DMAs (order = priority) ----------
            x_sb = wpool.tile([T, D], bf16, tag="x_sb")
            nc.sync.dma_start(x_sb[:], x_d[:, :])
            rows_g = wpool.tile([1, NL, F], bf16, tag="rows_g")
            nc.sync.dma_start(rows_g[:], g4_d.rearrange("r n -> () r n"))
            rows_sz = wpool.tile([1, 2 * NL, D], bf16, tag="rows_sz")
            nc.sync.dma_start(rows_sz[:, 0:NL, :],
                              s4_d.rearrange("r n -> () r n"))
            nc.sync.dma_start(rows_sz[:, NL:2 * NL, :],
                              zdb_d.rearrange("r n -> () r n"))

            winT_sb = wpool.tile([128, 6, F], bf16, tag="winT_sb")
            for k in range(6):
                nc.sync.dma_start(winT_sb[:, k, :],
                                  winT_d[128 * k:128 * (k + 1), :])
            woutT_sb = wpool.tile([128, 12, D], bf16, tag="woutT_sb")
            for g in range(4):
                nc.sync.dma_start(
                    woutT_sb[:, 3 * g:3 * (g + 1), :],
                    woutT_d[384 * g:384 * (g + 1), :].rearrange(
                        "(k p) n -> p k n", p=128))

            # ---------- broadcast tiles via K=1 ones-matmul ----------
            def bcast_build(dst, row_ap, chunks, eng_copy, label):
                for ci, (off, nn) in enumerate(chunks):
                    if nn == 512:
                        pt = ps_u.tile([T, 512], f32, tag=f"u{ci}",
                                       name=f"bc_{label}_{ci}")
                    else:
                        pt = ps_z.tile([T, nn], f32, tag=f"z{ci}",
                                       name=f"bc_{label}_{ci}")
                    nc.tensor.matmul(pt[:], ones1[:, :],
                                     row_ap[:, off:off + nn],
                                     start=True, stop=True)
                    eng_copy(dst[:, off:off + nn], pt[:])

            Sb, Gb = [], []
            for i in range(NL):
                sb = wpool.tile([T, D], bf16, tag=f"Sb{i}", name=f"Sb{i}")
                eng = nc.vector.tensor_copy if i == 0 else nc.scalar.copy
                bcast_build(sb, rows_sz[:, i, :], Z_CHUNKS, eng, f"sb{i}")
                Sb.append(sb)
            for m in range(NL):
                gb = wpool.tile([T, F], bf16, tag=f"Gb{m}", name=f"Gb{m}")
                eng = (nc.vector.tensor_copy if m in (0, 1)
                       else nc.scalar.copy)
                bcast_build(gb, rows_g[:, m, :], F_CHUNKS, eng, f"gb{m}")
                Gb.append(gb)

            # ---------- h0 = x + Sb0; transpose h0 on PE ----------
            h = work.tile([T, D], bf16, tag="h", bufs=2)
            nc.vector.tensor_add(h[:], x_sb[:], Sb[0][:])

            def transpose_to_sbuf(src, label, splits):
                """PE-transpose src [T, 768] -> [T, 768] bf16 k-tile major,
                one-bank bf16 PSUM; ACT evacuates per 3-tile half (the DVE
                is busy with the norm chain at that moment)."""
                t_ps = ps_t.tile([T, 6, 128], bf16, tag="t",
                                 name=f"tps_{label}")
                for lo, hi in splits:
                    for k in range(lo, hi):
                        nc.tensor.transpose(
                            t_ps[:, k, :],
                            src[:, 128 * k:128 * (k + 1)],
                            ident[:],
                        )
                t_sb = work.tile([T, D], bf16, tag="pT_sb", bufs=2,
                                 name=f"tsb_{label}")
                for half in range(2):
                    nc.scalar.copy(
                        t_sb[:, 384 * half:384 * (half + 1)],
                        t_ps[:, 3 * half:3 * (half + 1), :])
                return t_sb

            pT_sb = transpose_to_sbuf(h, "h0", ((0, 6),))

            accs = {}
            for j in (1, 2, 3):
                accs[j] = apool.tile([T, F], bf16, tag=f"acc{j}",
                                     name=f"acc{j}")

            rs_w = None
            # ---------- main loop ----------
            for i in range(NL):
                # MM1: p = (h|w) @ W_in^T. Loop 0 is winT-DMA-paced:
                # k-major. Steady loops: chunk-major for early retirement.
                u_ps = [ps_u.tile([T, 512], f32, tag=f"u{n}",
                                  name=f"u{i}_{n}") for n in range(3)]
                mm1_order = (
                    [(n, k) for k in range(6) for n in range(3)] if i == 0
                    else [(n, k) for n in range(3) for k in range(6)])
                for n, k in mm1_order:
                    nc.tensor.matmul(
                        u_ps[n][:],
                        pT_sb[:, 128 * k:128 * (k + 1)],
                        winT_sb[:, k, 512 * n:512 * (n + 1)],
                        start=(k == 0), stop=(k == 5),
                    )

                # u' = rs_w * p rides the ACT evacuation (plain copy at
                # i=0); y combine + per-chunk PE transposes follow
                u_sb = work.tile([T, F], bf16, tag="u_sb", bufs=2)
                y = work.tile([T, F], bf16, tag="y", bufs=2)
                yT_ps = ps_y.tile([T, 12, 128], bf16, tag="yt")
                yT_sb = work.tile([128, 12, 128], bf16, tag="yT_sb", bufs=2)
                for n in range(3):
                    sl = slice(512 * n, 512 * (n + 1))
                    if i == 0:
                        nc.scalar.copy(u_sb[:, sl], u_ps[n][:])
                    else:
                        nc.scalar.activation(u_sb[:, sl], u_ps[n][:],
                                             AF.Copy, scale=rs_w[:, :])
                    nc.vector.tensor_mul(y[:, sl], u_sb[:, sl], Gb[0][:, sl])
                    if i > 0:
                        nc.vector.tensor_add(y[:, sl], y[:, sl],
                                             accs[i][:, sl])
                    for c in range(4 * n, 4 * (n + 1)):
                        nc.tensor.transpose(
                            yT_ps[:, c, :],
                            y[:, 128 * c:128 * (c + 1)],
                            ident[:],
                        )
                    nc.vector.tensor_copy(yT_sb[:, 4 * n:4 * (n + 1), :],
                                          yT_ps[:, 4 * n:4 * (n + 1), :])

                # MM2: z = y @ out_proj^T (+ ones x zdb_i), A(512), B(256)
                z_ps = []
                for ci, (off, nn) in enumerate(Z_CHUNKS):
                    zt = ps_z.tile([T, nn], f32, tag=f"z{ci}",
                                   name=f"z{i}_{ci}")
                    if i > 0:
                        nc.tensor.matmul(
                            zt[:], ones1[:, :],
                            rows_sz[:, NL + i, off:off + nn],
                            start=True, stop=False)
                    for c in range(12):
                        nc.tensor.matmul(
                            zt[:],
                            yT_sb[:, c, :],
                            woutT_sb[:, c, off:off + nn],
                            start=(c == 0 and i == 0), stop=(c == 11),
                        )
                    z_ps.append(zt)

                # norm: A partial on ACT under B's matmuls; B partial on
                # DVE at B-stop (fused square+accumulate), add + Rsqrt on
                # ACT. The whole chain feeds w as fast as possible.
                ssz_A = scal.tile([T, 1], f32, tag="sszA")
                scrA = work.tile([T, 512], bf16, tag="scr5", bufs=2)
                nc.scalar.activation(scrA[:], z_ps[0][:], AF.Square,
                                     accum_out=ssz_A[:])
                ssz_B = scal.tile([T, 1], f32, tag="sszB")
                scrB = work.tile([T, 512], bf16, tag="zscr", bufs=2)
                nc.scalar.activation(scrB[:, 0:256], z_ps[1][:], AF.Square,
                                     accum_out=ssz_B[:])
                ss_z = scal.tile([T, 1], f32, tag="ss_z")
                nc.scalar.activation(ss_z[:], ssz_A[:], AF.Identity,
                                     bias=ssz_B[:, :])
                rs_z = scal.tile([T, 1], f32, tag="rs_z")
                _act_rsqrt(nc, rs_z[:], ss_z[:], eps_t[:, :], 1.0 / D)

                # w = z * rs_z + h (two chunks feeding the PE transposes)
                w = work.tile([T, D], bf16, tag="w", bufs=2)
                for ci, (off, nn) in enumerate(Z_CHUNKS):
                    nc.vector.scalar_tensor_tensor(
                        out=w[:, off:off + nn], in0=z_ps[ci][:],
                        scalar=rs_z[:, :], in1=h[:, off:off + nn],
                        op0=AL.mult, op1=AL.add,
                    )
                if i < NL - 1:
                    pT_sb = transpose_to_sbuf(w, f"w{i}",
                                              ((0, 4), (4, 6)))

                # ss_w on DVE (fused square+accum), Rsqrt on ACT right
                # after the transpose evacuations so rs_w beats the next
                # loop's u' evacuation
                ss_w = scal.tile([T, 1], f32, tag="ss_w")
                scrw = work.tile([T, D], bf16, tag="scrw", bufs=2)
                nc.vector.scalar_tensor_tensor(
                    out=scrw[:], in0=w[:], scalar=1.0, in1=w[:],
                    op0=AL.mult, op1=AL.mult, accum_out=ss_w[:],
                )
                rs_w = scal.tile([T, 1], f32, tag="rs_w", bufs=2,
                                 name=f"rs_w{i}")
                _act_rsqrt(nc, rs_w[:], ss_w[:], eps_t[:, :], 1.0 / D)

                if i < NL - 1:
                    h_next = work.tile([T, D], bf16, tag="h", bufs=2)
                    nc.vector.scalar_tensor_tensor(
                        out=h_next[:], in0=w[:], scalar=rs_w[:, :],
                        in1=Sb[i + 1][:], op0=AL.mult, op1=AL.add,
                    )
                    h = h_next
                else:
                    out_f = work.tile([T, D], f32, tag="out_f", bufs=1)
                    nc.vector.tensor_scalar_mul(out_f[:], w[:], rs_w[:, :])
                    nc.sync.dma_start(out_d[:, :], out_f[:])

                # acc updates last: muls on DVE, the cross-loop += on
                # GpSimd (deep slack, keeps the DVE tail free)
                for j in range(i + 1, NL):
                    m = j - i
                    if i == 0:
                        nc.vector.tensor_mul(accs[j][:], u_sb[:], Gb[m][:])
                    else:
                        tmp_a = work.tile([T, F], bf16, tag="tmp_a", bufs=2)
                        nc.vector.tensor_mul(tmp_a[:], u_sb[:], Gb[m][:])
                        nc.gpsimd.tensor_add(accs[j][:], accs[j][:],
                                             tmp_a[:])

    nc.compile()
    return nc


def _host_prep(x, in_proj_base, lora_A, lora_B, A_theta, B_real, B_imag,
               C_real, C_imag, out_proj_w, step_emb):
    W_in = in_proj_base.astype(np.float64) + 2.0 * (
        lora_B.astype(np.float64) @ lora_A.astype(np.float64))
    winT = np.ascontiguousarray(W_in.T).astype(ml_dtypes.bfloat16)
    woutT = np.ascontiguousarray(out_proj_w.T).astype(ml_dtypes.bfloat16)

    th = A_theta.astype(np.float64)
    P = (C_real.astype(np.float64) * B_real.astype(np.float64)
         - C_imag.astype(np.float64) * B_imag.astype(np.float64))
    Q = (C_real.astype(np.float64) * B_imag.astype(np.float64)
         + C_imag.astype(np.float64) * B_real.astype(np.float64))
    g4_f64 = np.stack([
        (P * np.cos(m * th) - Q * np.sin(m * th)).sum(-1).reshape(-1)
        for m in range(NL)
    ])                                                       # [4, 1536]
    g4 = g4_f64.astype(ml_dtypes.bfloat16)
    s4 = np.ascontiguousarray(step_emb).astype(ml_dtypes.bfloat16)

    # sW_k = step_emb[k] @ W_in^T; zdb_j = (sum_{k=1..j} G_{j-k}*sW_k)
    # @ W_out^T: constant rank-1 rows injected into MM2's PSUM. Loop 0
    # has no step contribution (h0 = x + s0 explicit).
    sW = step_emb.astype(np.float64) @ W_in.T                # [4, F]
    zdb = np.zeros((NL, D))
    for j in range(1, NL):
        db_j = np.zeros(F)
        for k in range(1, j + 1):
            db_j += g4_f64[j - k] * sW[k]
        zdb[j] = db_j @ out_proj_w.astype(np.float64).T
    return winT, woutT, g4, s4, zdb.astype(ml_dtypes.bfloat16)


def kernel(x, in_proj_base, lora_A, lora_B, A_theta, B_real, B_imag,
           C_real, C_imag, out_proj_w, mixer_norm_w, loop_norm_w, step_emb,
           _trace=False):
    x = np.asarray(x, dtype=np.float32)
    winT, woutT, g4, s4, zdb = _host_prep(
        np.asarray(x), np.asarray(in_proj_base), np.asarray(lora_A),
        np.asarray(lora_B), np.asarray(A_theta), np.asarray(B_real),
        np.asarray(B_imag), np.asarray(C_real), np.asarray(C_imag),
        np.asarray(out_proj_w), np.asarray(step_emb))
    # mixer_norm_w / loop_norm_w are ones per the problem spec; rmsnorm weight
    # multiplies are identity and omitted on device.

    if "nc" not in _CACHE:
        _CACHE["nc"] = build_nc()
    nc = _CACHE["nc"]

    x16 = x.astype(ml_dtypes.bfloat16)
    shared = {"winT": winT, "woutT": woutT, "g4": g4, "s4": s4, "zdb": zdb}
    in_maps = [
        {**shared, "x_in": np.ascontiguousarray(x16[0, T * c:T * (c + 1), :])}
        for c in range(NCORES)
    ]
    res = run_bass_kernel_spmd(nc, in_maps, list(range(NCORES)), trace=_trace)
    out = np.concatenate(
        [np.asarray(res.results[c]["x_out"]) for c in range(NCORES)], axis=0)
    if _trace:
        _CACHE["last_result"] = res
    return out[None, :, :].astype(np.float32)
